# revision 1
# baseline (speedup 1.0000x reference)
"""Self-contained GAT (3-layer, 2-head) kernel for Trainium2, 8 NeuronCores.

Destination-sharded GAT: per-layer node-feature table built on device and
AllGathered; per-edge rows fetched with [P,1] indirect DMA; segment softmax
folded into a ratio of two one-hot PE-matmul segment sums; per-layer programs
launched sequentially, then pooling + MLP.
"""
"""GAT Trainium kernel: host prep + bass builder + runner. See memory/gat-kernel-design."""
import sys
sys.path.insert(0, '/opt/trn_rl_repo')
import numpy as np
import concourse.bass as bass
import concourse.bacc as bacc
import concourse.mybir as mybir
from concourse.tile import TileContext
from concourse.bass_utils import run_bass_kernel_spmd

P = 128
WIN = 64
TCOLS = 68           # [h0(0:32),1@32,h1(33:65),1@65,as0@66,as1@67]
NEG = 0.2
F32 = mybir.dt.float32
U8 = mybir.dt.uint8
I32 = mybir.dt.int32


def prep_core(src, dst, n0, n_local, GPW):
    """Slot structure for one core; groups of 128 edges, each within one
    64-node window; every window padded to exactly GPW groups."""
    m = (dst >= n0) & (dst < n0 + n_local)
    es = src[m].astype(np.int64)
    ed = (dst[m] - n0).astype(np.int64)
    o = np.argsort(ed, kind='stable')
    es, ed = es[o], ed[o]
    nwin = (n_local + WIN - 1) // WIN
    NG = nwin * GPW
    idx = np.zeros((NG, P), dtype=np.int32)          # src (pad -> 0)
    rel = np.full((NG, P), 255.0, dtype=np.float32)  # dst - w0 (pad -> 255)
    wstart = np.searchsorted(ed, np.arange(nwin + 1) * WIN)
    for w in range(nwin):
        lo, hi = wstart[w], wstart[w + 1]
        cnt = hi - lo
        assert cnt <= GPW * P, f"window {w}: {cnt} edges > {GPW * P}"
        g0 = w * GPW
        for j in range((cnt + P - 1) // P):
            a = lo + j * P
            b = min(a + P, hi)
            idx[g0 + j, :b - a] = es[a:b]
            rel[g0 + j, :b - a] = ed[a:b] - w * WIN
    # oht8: per window-pair [128, GPW, 128] u8; rows 0:64 even win, 64:128 odd
    npair = (nwin + 1) // 2
    oht8 = np.zeros((npair, 2, GPW, WIN, P), dtype=np.uint8)
    kk = np.arange(WIN)
    for w in range(nwin):
        for j in range(GPW):
            r = rel[w * GPW + j]
            oht8[w // 2, w % 2, j] = (r[None, :] == kk[:, None])
    # reorder to [128 part(2*WIN), npair*GPW*128]
    oht8 = oht8.transpose(0, 2, 1, 3, 4).reshape(npair, GPW, 2 * WIN, P)
    oht8 = oht8.transpose(2, 0, 1, 3).reshape(2 * WIN, npair * GPW * P)
    return dict(idx=np.ascontiguousarray(idx.T), rel=np.ascontiguousarray(rel.T),
                oht8=np.ascontiguousarray(oht8), nwin=nwin, NG=NG)


def compute_gpw(src, dst, n_cores, N):
    n_local = N // n_cores
    gpw = 0
    for c in range(n_cores):
        m = (dst >= c * n_local) & (dst < (c + 1) * n_local)
        ed = np.sort(dst[m] - c * n_local)
        ws = np.searchsorted(ed, np.arange((n_local + WIN - 1) // WIN + 1) * WIN)
        cnt = np.diff(ws)
        gpw = max(gpw, int(np.max((cnt + P - 1) // P)))
    return gpw


def wtr_layout(WT):
    """[fin, 64] -> [fin, 68] with layout cols + zero slots."""
    fin = WT.shape[0]
    out = np.zeros((fin, TCOLS), dtype=np.float32)
    out[:, 0:32] = WT[:, 0:32]
    out[:, 33:65] = WT[:, 32:64]
    return out


def prep_all(inputs, n_cores, N, G, HC, F_IN):
    E = np.asarray(inputs['edge_index']).shape[1]
    loops = np.arange(N, dtype=np.int64)
    src = np.concatenate([np.asarray(inputs['edge_index'][0]).astype(np.int64), loops])
    dst = np.concatenate([np.asarray(inputs['edge_index'][1]).astype(np.int64), loops])
    n_local = N // n_cores
    GPW = compute_gpw(src, dst, n_cores, N)
    batch = np.asarray(inputs['batch']).astype(np.int64)
    counts = np.bincount(batch, minlength=G).astype(np.float32).reshape(G, 1)
    x = np.asarray(inputs['x']).astype(np.float32)
    cores = []
    nch = (n_local + P - 1) // P
    for c in range(n_cores):
        d = prep_core(src, dst, c * n_local, n_local, GPW)
        d['x_slice'] = np.ascontiguousarray(x[c * n_local:(c + 1) * n_local])
        b_loc = batch[c * n_local:(c + 1) * n_local]
        pg = np.zeros((nch, P, G), dtype=np.float32)
        for ch in range(nch):
            bb = b_loc[ch * P:(ch + 1) * P]
            pg[ch, np.arange(len(bb)), bb] = 1.0
        d['pg'] = pg.reshape(nch, P, G).transpose(1, 0, 2).reshape(P, nch * G)
        d['pg'] = np.ascontiguousarray(d['pg'])
        cores.append(d)
    H = 2
    Cc = HC // H

    def avec(a_s, a_d):
        A = np.zeros((HC, 4), dtype=np.float32)
        for h in range(H):
            A[h * Cc:(h + 1) * Cc, h] = np.asarray(a_s).reshape(H, Cc)[h]
            A[h * Cc:(h + 1) * Cc, 2 + h] = np.asarray(a_d).reshape(H, Cc)[h]
        return A
    Ws = [np.asarray(inputs['W0']).astype(np.float32),
          np.asarray(inputs['W1']).astype(np.float32),
          np.asarray(inputs['W2']).astype(np.float32)]
    meta = dict(
        n_local=n_local, GPW=GPW, counts=counts, G=G, F_IN=F_IN, HC=HC, N=N,
        n_cores=n_cores,
        W=[np.ascontiguousarray(w) for w in Ws],
        WTr=[np.ascontiguousarray(wtr_layout(w.T)) for w in Ws],
        Avec=[avec(inputs['a_src0'], inputs['a_dst0']),
              avec(inputs['a_src1'], inputs['a_dst1']),
              avec(inputs['a_src2'], inputs['a_dst2'])],
        mlp_w1T=np.ascontiguousarray(np.asarray(inputs['mlp_w1']).T).astype(np.float32),
        mlp_w2T=np.ascontiguousarray(np.asarray(inputs['mlp_w2']).T).astype(np.float32),
        b1rep=np.tile(np.asarray(inputs['mlp_b1']).astype(np.float32)[None, :], (G, 1)),
        b2rep=np.tile(np.asarray(inputs['mlp_b2']).astype(np.float32)[None, :], (G, 1)),
    )
    for l in range(3):
        assert np.all(np.asarray(inputs[f'b{l}']) == 0), "nonzero GAT bias unsupported"
    return cores, meta


def build_layer(meta, fin, nwin, NG):
    n_local, GPW, G = meta['n_local'], meta['GPW'], meta['G']
    HC, N = meta['HC'], meta['N']
    n_cores = meta['n_cores']
    nch = (n_local + P - 1) // P
    npair = (nwin + 1) // 2
    AF = mybir.ActivationFunctionType
    OP = mybir.AluOpType

    nc = bacc.Bacc("TRN2", target_bir_lowering=False, debug=False,
                   num_devices=n_cores)
    # inputs
    t_xsrc = nc.dram_tensor("xsrc", [n_local, fin], F32, kind="ExternalInput")
    t_idx = nc.dram_tensor("idx", [P, NG], I32, kind="ExternalInput")
    t_rel = nc.dram_tensor("rel", [P, NG], F32, kind="ExternalInput")
    t_oht = nc.dram_tensor("oht8", [2 * WIN, npair * GPW * P], U8, kind="ExternalInput")
    t_Wl = nc.dram_tensor("W", [HC, fin], F32, kind="ExternalInput")
    t_WTrl = nc.dram_tensor("WTr", [fin, TCOLS], F32, kind="ExternalInput")
    t_Avl = nc.dram_tensor("Avec", [HC, 4], F32, kind="ExternalInput")
    t_xn = nc.dram_tensor("xn", [n_local, HC], F32, kind="ExternalOutput")
    with TileContext(nc) as tc:
        from concourse.masks import make_identity
        from contextlib import ExitStack
        with ExitStack() as ctx:
            const = ctx.enter_context(tc.tile_pool(name="const", bufs=1))
            dram = ctx.enter_context(tc.tile_pool(name="dram", bufs=2, space="DRAM"))
            dram1 = ctx.enter_context(tc.tile_pool(name="dram1", bufs=1, space="DRAM"))
            sb = ctx.enter_context(tc.tile_pool(name="sb", bufs=3))
            sg = ctx.enter_context(tc.tile_pool(name="sg", bufs=2))     # big gather tiles
            sw = ctx.enter_context(tc.tile_pool(name="sw", bufs=2))     # S tiles
            ps = ctx.enter_context(tc.tile_pool(name="ps", bufs=1, space="PSUM"))
            psw = ctx.enter_context(tc.tile_pool(name="psw", bufs=2, space="PSUM"))
            psa = ctx.enter_context(tc.tile_pool(name="psa", bufs=2, space="PSUM"))

            ident = const.tile([P, P], F32)
            make_identity(nc, ident[:])
            zero128 = const.tile([P, P], F32)
            nc.vector.memset(zero128[:], 0.0)
            zero66 = const.tile([P, 66], F32)
            nc.vector.memset(zero66[:], 0.0)
            ones1 = const.tile([P, 1], F32)
            nc.vector.memset(ones1[:], 1.0)
            iota64i = const.tile([P, WIN], I32)
            nc.gpsimd.iota(iota64i[:], pattern=[[1, WIN]], base=0, channel_multiplier=0)
            iota64 = const.tile([P, WIN], F32)
            nc.vector.tensor_copy(iota64[:], iota64i[:])

            if True:
                # ---- Wcat [fin, 70] ----
                Wl = sb.tile([HC, fin], F32)
                nc.sync.dma_start(out=Wl[:], in_=t_Wl.ap())
                Av = sb.tile([HC, 4], F32)
                nc.sync.dma_start(out=Av[:], in_=t_Avl.ap())
                ps_a = ps.tile([fin, 4], F32, space="PSUM", tag="ps_a")
                nc.tensor.matmul(out=ps_a[:], lhsT=Wl[:], rhs=Av[:], start=True, stop=True)
                Wcat = sb.tile([fin, TCOLS + 2], F32)
                nc.sync.dma_start(out=Wcat[:, 0:TCOLS], in_=t_WTrl.ap())
                nc.scalar.copy(Wcat[:, 66:70], ps_a[:])
                # ---- table slice build ----
                slice_t = dram.tile([n_local, TCOLS], F32)
                adloc = dram.tile([n_local, 2], F32)
                xsrc_ap = t_xsrc.ap()
                for c in range(nch):
                    pc = min(P, n_local - c * P)
                    xc = sb.tile([P, fin], F32, tag="xc")
                    nc.sync.dma_start(out=xc[:pc], in_=xsrc_ap[c * P:c * P + pc, :])
                    ps_t = ps.tile([fin, P], F32, space="PSUM", tag="ps_tr")
                    nc.tensor.transpose(out=ps_t[:, 0:pc], in_=xc[:pc], identity=ident[:pc, :pc])
                    xT = sb.tile([fin, P], F32, tag="xT")
                    nc.scalar.copy(xT[:, 0:pc], ps_t[:, 0:pc])
                    ps_r = ps.tile([P, TCOLS + 2], F32, space="PSUM", tag="ps_tr")
                    nc.tensor.matmul(out=ps_r[:pc, :], lhsT=xT[:, 0:pc], rhs=Wcat[:], start=True, stop=True)
                    tt = sb.tile([P, TCOLS + 2], F32, tag="tt")
                    nc.scalar.copy(tt[:pc, :], ps_r[:pc, :])
                    nc.vector.tensor_copy(tt[:pc, 32:33], ones1[:pc])
                    nc.vector.tensor_copy(tt[:pc, 65:66], ones1[:pc])
                    nc.sync.dma_start(out=slice_t[c * P:c * P + pc, :], in_=tt[:pc, 0:TCOLS])
                    nc.sync.dma_start(out=adloc[c * P:c * P + pc, :], in_=tt[:pc, TCOLS:TCOLS + 2])
                # ---- AllGather table ----
                table = dram.tile([N, TCOLS], F32)
                nc.gpsimd.collective_compute(
                    "AllGather", OP.bypass,
                    replica_groups=[list(range(n_cores))],
                    ins=[slice_t.opt()], outs=[table.opt()])
                # ---- gather + aggregate ----
                xn_new = t_xn.ap()
                for w in range(nwin):
                    if w % 2 == 0:
                        ohtc = sg.tile([2 * WIN, GPW * P], U8, tag="ohtc")
                        nc.sync.dma_start(
                            out=ohtc[:],
                            in_=t_oht.ap()[:, (w // 2) * GPW * P:(w // 2 + 1) * GPW * P])
                        ohtf = sg.tile([2 * WIN, GPW * P], F32, tag="ohtf")
                        nc.vector.tensor_copy(ohtf[:], ohtc[:])
                    wb = WIN * (w % 2)
                    nnode = min(WIN, n_local - w * WIN)
                    idxt = sb.tile([P, GPW], I32, tag="idxt")
                    nc.sync.dma_start(out=idxt[:], in_=t_idx.ap()[:, w * GPW:(w + 1) * GPW])
                    relt = sb.tile([P, GPW], F32, tag="relt")
                    nc.sync.dma_start(out=relt[:], in_=t_rel.ap()[:, w * GPW:(w + 1) * GPW])
                    adw = sb.tile([2 * WIN, 2], F32, tag="adw")
                    nc.vector.memset(adw[:], 0.0)
                    nc.sync.dma_start(out=adw[wb:wb + nnode, :],
                                      in_=adloc[w * WIN:w * WIN + nnode, :])
                    # gather
                    gt = sg.tile([P, GPW, TCOLS], F32, tag="gt")
                    for j in range(GPW):
                        nc.gpsimd.indirect_dma_start(
                            out=gt[:, j, :], out_offset=None,
                            in_=table[:],
                            in_offset=bass.IndirectOffsetOnAxis(ap=idxt[:, j:j + 1], axis=0))
                    # alpha_d expansion: per group MM -> psum [128, 2*GPW]
                    ps_ad = psa.tile([P, 2 * GPW], F32, space="PSUM", tag="ps_ad")
                    for j in range(GPW):
                        nc.tensor.matmul(
                            out=ps_ad[:, 2 * j:2 * j + 2],
                            lhsT=ohtf[wb:wb + WIN, j * P:(j + 1) * P],
                            rhs=adw[wb:wb + WIN, :], start=True, stop=True)
                    # e/w
                    ew = sb.tile([P, 2 * GPW], F32, tag="ew")
                    nc.vector.tensor_tensor(
                        out=ew[:].rearrange("p (g h) -> p g h", g=GPW),
                        in0=gt[:, :, 66:68],
                        in1=ps_ad[:].rearrange("p (g h) -> p g h", g=GPW), op=OP.add)
                    ew2 = sb.tile([P, 2 * GPW], F32, tag="ew2")
                    nc.vector.tensor_scalar_mul(ew2[:], ew[:], NEG)
                    nc.vector.tensor_tensor(out=ew[:], in0=ew[:], in1=ew2[:], op=OP.max)
                    nc.scalar.activation(ew[:], ew[:], AF.Exp)
                    # m and S2
                    mall = sw.tile([P, GPW * WIN], F32, tag="mall")
                    nc.vector.tensor_tensor(
                        out=mall[:].rearrange("p (g w) -> p g w", g=GPW),
                        in0=iota64[:].rearrange("p (u w) -> p u w", u=1).to_broadcast([P, GPW, WIN]),
                        in1=relt[:].rearrange("p (g u) -> p g u", u=1).to_broadcast([P, GPW, WIN]),
                        op=OP.is_equal)
                    S2 = sw.tile([P, GPW * 2 * WIN], F32, tag="S2")
                    nc.vector.tensor_tensor(
                        out=S2[:].rearrange("p (g h w) -> p g h w", g=GPW, h=2),
                        in0=mall[:].rearrange("p (g u w) -> p g u w", g=GPW, u=1).to_broadcast([P, GPW, 2, WIN]),
                        in1=ew[:].rearrange("p (g h u) -> p g h u", g=GPW, u=1).to_broadcast([P, GPW, 2, WIN]),
                        op=OP.mult)
                    # aggregation
                    ps_n = psw.tile([P, 66], F32, space="PSUM", tag="ps_n")
                    nc.tensor.matmul(out=ps_n[:], lhsT=zero128[:], rhs=zero66[:],
                                     start=True, stop=False)
                    for j in range(GPW):
                        nc.tensor.matmul(
                            out=ps_n[:], lhsT=S2[:, j * 2 * WIN:(j + 1) * 2 * WIN],
                            rhs=gt[:, j, 0:66],
                            start=False, stop=(j == GPW - 1))
                    # epilogue (per head half)
                    for h in (0, 1):
                        rows = slice(WIN * h, WIN * h + nnode)
                        c0 = 33 * h
                        den = sb.tile([WIN, 1], F32, tag=f"den{h}")
                        nc.vector.tensor_scalar_add(den[:nnode], ps_n[rows, c0 + 32:c0 + 33], 1e-16)
                        rec = sb.tile([WIN, 1], F32, tag=f"rec{h}")
                        nc.vector.reciprocal(rec[:nnode], den[:nnode])
                        hv = sb.tile([WIN, 32], F32, tag=f"hv{h}")
                        nc.vector.tensor_tensor(out=hv[:nnode], in0=ps_n[rows, c0:c0 + 32],
                                                in1=rec[:nnode].to_broadcast([nnode, 32]),
                                                op=OP.mult)
                        t1 = sb.tile([WIN, 32], F32, tag=f"t1{h}")
                        nc.vector.tensor_scalar_max(t1[:nnode], hv[:nnode], 0.0)
                        t2 = sb.tile([WIN, 32], F32, tag=f"t2{h}")
                        nc.vector.tensor_scalar_min(t2[:nnode], hv[:nnode], 0.0)
                        nc.scalar.activation(t2[:nnode], t2[:nnode], AF.Exp)
                        nc.vector.tensor_tensor(out=t1[:nnode], in0=t1[:nnode], in1=t2[:nnode], op=OP.add)
                        nc.vector.tensor_scalar_add(t1[:nnode], t1[:nnode], -1.0)
                        nc.sync.dma_start(out=xn_new[w * WIN:w * WIN + nnode, 32 * h:32 * h + 32],
                                          in_=t1[:nnode, :])
    nc.compile()
    return nc

def build_pool(meta):
    n_local, G, HC = meta['n_local'], meta['G'], meta['HC']
    n_cores = meta['n_cores']
    nch = (n_local + P - 1) // P
    OP = mybir.AluOpType
    AF = mybir.ActivationFunctionType
    nc = bacc.Bacc("TRN2", target_bir_lowering=False, debug=False, num_devices=n_cores)
    t_xsrc = nc.dram_tensor("xsrc", [n_local, HC], F32, kind="ExternalInput")
    t_pg = nc.dram_tensor("pg", [P, nch * G], F32, kind="ExternalInput")
    t_cnt = nc.dram_tensor("counts", [G, 1], F32, kind="ExternalInput")
    t_w1T = nc.dram_tensor("mlp_w1T", [HC, 32], F32, kind="ExternalInput")
    t_w2T = nc.dram_tensor("mlp_w2T", [32, 2], F32, kind="ExternalInput")
    t_b1 = nc.dram_tensor("b1rep", [G, 32], F32, kind="ExternalInput")
    t_b2 = nc.dram_tensor("b2rep", [G, 2], F32, kind="ExternalInput")
    t_out = nc.dram_tensor("out", [G, 2], F32, kind="ExternalOutput")
    with TileContext(nc) as tc:
        from concourse.masks import make_identity
        from contextlib import ExitStack
        with ExitStack() as ctx:
            const = ctx.enter_context(tc.tile_pool(name="const", bufs=1))
            dram1 = ctx.enter_context(tc.tile_pool(name="dram1", bufs=1, space="DRAM"))
            sb = ctx.enter_context(tc.tile_pool(name="sb", bufs=3))
            ps = ctx.enter_context(tc.tile_pool(name="ps", bufs=1, space="PSUM"))
            ident = const.tile([P, P], F32)
            make_identity(nc, ident[:])
            zero128 = const.tile([P, P], F32)
            nc.vector.memset(zero128[:], 0.0)
            ps_g = ps.tile([G, HC], F32, space="PSUM", tag="ps_g")
            nc.tensor.matmul(out=ps_g[:], lhsT=zero128[:, 0:G], rhs=zero128[:, 0:HC],
                             start=True, stop=False)
            for c in range(nch):
                pc = min(P, n_local - c * P)
                xc = sb.tile([P, HC], F32, tag="xc2")
                nc.sync.dma_start(out=xc[:pc], in_=t_xsrc.ap()[c * P:c * P + pc, :])
                pgt = sb.tile([P, G], F32, tag="pgt")
                nc.sync.dma_start(out=pgt[:], in_=t_pg.ap()[:, c * G:(c + 1) * G])
                nc.tensor.matmul(out=ps_g[:], lhsT=pgt[:pc, :], rhs=xc[:pc, :],
                                 start=False, stop=(c == nch - 1))
            pool_l = dram1.tile([G, HC], F32)
            pool_s = sb.tile([G, HC], F32)
            nc.scalar.copy(pool_s[:], ps_g[:])
            nc.sync.dma_start(out=pool_l[:], in_=pool_s[:])
            pool_r = dram1.tile([G, HC], F32)
            nc.gpsimd.collective_compute(
                "AllReduce", mybir.AluOpType.add,
                replica_groups=[list(range(n_cores))],
                ins=[pool_l.opt()], outs=[pool_r.opt()])
            pooled = sb.tile([G, HC], F32)
            nc.sync.dma_start(out=pooled[:], in_=pool_r[:])
            cnt = sb.tile([G, 1], F32)
            nc.sync.dma_start(out=cnt[:], in_=t_cnt.ap())
            nc.vector.tensor_scalar_max(cnt[:], cnt[:], 1.0)
            rc = sb.tile([G, 1], F32)
            nc.vector.reciprocal(out=rc[:], in_=cnt[:])
            nc.vector.tensor_tensor(out=pooled[:], in0=pooled[:],
                                    in1=rc[:].to_broadcast([G, HC]), op=OP.mult)
            # MLP
            ps_pt = ps.tile([HC, G], F32, space="PSUM", tag="mlp_ps")
            nc.tensor.transpose(out=ps_pt[:], in_=pooled[:], identity=ident[0:G, 0:G])
            poolT = sb.tile([HC, G], F32)
            nc.scalar.copy(poolT[:], ps_pt[:])
            w1 = sb.tile([HC, 32], F32)
            nc.sync.dma_start(out=w1[:], in_=t_w1T.ap())
            ps_z = ps.tile([G, 32], F32, space="PSUM", tag="mlp_ps")
            nc.tensor.matmul(out=ps_z[:], lhsT=poolT[:], rhs=w1[:], start=True, stop=True)
            z1 = sb.tile([G, 32], F32)
            b1t = sb.tile([G, 32], F32)
            nc.sync.dma_start(out=b1t[:], in_=t_b1.ap())
            nc.vector.tensor_tensor(out=z1[:], in0=ps_z[:], in1=b1t[:], op=OP.add)
            nc.scalar.activation(z1[:], z1[:], AF.Relu)
            ps_zt = ps.tile([32, G], F32, space="PSUM", tag="mlp_ps")
            nc.tensor.transpose(out=ps_zt[:], in_=z1[:], identity=ident[0:G, 0:G])
            z1T = sb.tile([32, G], F32)
            nc.scalar.copy(z1T[:], ps_zt[:])
            w2 = sb.tile([32, 2], F32)
            nc.sync.dma_start(out=w2[:], in_=t_w2T.ap())
            ps_o = ps.tile([G, 2], F32, space="PSUM", tag="mlp_ps")
            nc.tensor.matmul(out=ps_o[:], lhsT=z1T[:], rhs=w2[:], start=True, stop=True)
            b2t = sb.tile([G, 2], F32)
            nc.sync.dma_start(out=b2t[:], in_=t_b2.ap())
            outt = sb.tile([G, 2], F32)
            nc.vector.tensor_tensor(out=outt[:], in0=ps_o[:], in1=b2t[:], op=OP.add)
            nc.sync.dma_start(out=t_out.ap(), in_=outt[:])
    nc.compile()
    return nc




def _in_maps_layer(cores, meta, l, xn_slices, n_cores):
    ims = []
    for c in range(n_cores):
        d = cores[c]
        im = dict(idx=d['idx'], rel=d['rel'], oht8=d['oht8'],
                  W=meta['W'][l], WTr=meta['WTr'][l], Avec=meta['Avec'][l])
        im['xsrc'] = d['x_slice'] if l == 0 else xn_slices[c]
        ims.append(im)
    return ims


def run(inputs, N, G, HC, F_IN, n_cores=8, trace=False):
    cores, meta = prep_all(inputs, n_cores, N, G, HC, F_IN)
    meta['F_IN'] = F_IN
    nwin, NG = cores[0]['nwin'], cores[0]['NG']
    nc0 = build_layer(meta, F_IN, nwin, NG)
    ncm = build_layer(meta, HC, nwin, NG)
    ncp = build_pool(meta)
    total_ns = 0
    xn = None
    for l in range(3):
        nc_l = nc0 if l == 0 else ncm
        res = run_bass_kernel_spmd(nc_l, _in_maps_layer(cores, meta, l, xn, n_cores),
                                   core_ids=list(range(n_cores)), trace=trace)
        xn = [np.asarray(res.results[c]['xn']) for c in range(n_cores)]
        if trace and res.exec_time_ns:
            total_ns += res.exec_time_ns
    ims = []
    for c in range(n_cores):
        ims.append(dict(xsrc=xn[c], pg=cores[c]['pg'], counts=meta['counts'],
                        mlp_w1T=meta['mlp_w1T'], mlp_w2T=meta['mlp_w2T'],
                        b1rep=meta['b1rep'], b2rep=meta['b2rep']))
    res = run_bass_kernel_spmd(ncp, ims, core_ids=list(range(n_cores)), trace=trace)
    if trace and res.exec_time_ns:
        total_ns += res.exec_time_ns

    class R:
        exec_time_ns = total_ns if trace else None
    return np.asarray(res.results[0]['out']), R


N_FULL, F_IN_FULL, H_FULL, C_FULL, E_FULL, G_FULL = 100000, 128, 2, 32, 3200000, 64
HC_FULL = H_FULL * C_FULL


def kernel(**inputs):
    import os
    trace = bool(os.environ.get("GAT_TRACE"))
    out, res = run(inputs, N_FULL, G_FULL, HC_FULL, F_IN_FULL, n_cores=8, trace=trace)
    if trace:
        kernel.last_exec_ns = res.exec_time_ns
    return np.asarray(out, dtype=np.float32)



# revision 8
# speedup vs baseline: 1.2980x; 1.2980x over previous
"""Self-contained 3-layer GAT kernel for Trainium2, 8 NeuronCores.

Destination-sharded GAT. Per layer: build a bf16 node table [N, 128]
(h 64 | alpha_src 2 | 1.0 | alpha_dst 2 | pad) on device, AllGather it,
then per 64-node dst window gather per-edge source rows with the GPSIMD
dma_gather custom op (int16 indices, 4 src-range bins of 32768 rows,
256B rows), compute segment softmax via exp-weighted one-hot bf16 PE
matmuls (denominator via the packed ones column), ELU, and write xn.
Pool + MLP fused after layer 2.  Single program, single launch.
"""
import sys
sys.path.insert(0, '/opt/trn_rl_repo')
import numpy as np
import ml_dtypes
import concourse.bass as bass
import concourse.bacc as bacc
import concourse.mybir as mybir
from concourse.tile import TileContext
from concourse.bass_utils import run_bass_kernel_spmd
from contextlib import ExitStack

P = 128
WIN = 64
EC = 128             # table row cols (bf16) = 256B
NBIN = 4
BINSZ = 32768
WB = 4               # windows per gather batch
KEPI = 14            # windows per epilogue batch
NEG = 0.2
F32 = mybir.dt.float32
BF16 = mybir.dt.bfloat16
U8 = mybir.dt.uint8
I16 = mybir.dt.int16
BF = ml_dtypes.bfloat16

N_FULL, F_IN, H, C, G_FULL = 100000, 128, 2, 32, 64
HC = H * C
N_CORES = 8
NL = N_FULL // N_CORES          # 12500
NCH = (NL + P - 1) // P         # 98
NLP = NCH * P                   # 12544
NWIN = (NL + WIN - 1) // WIN    # 196
NBATCH = NWIN // WB             # 49


# ---------------------------------------------------------------- host prep

def build_structure(src, dst):
    """Global static structure: per (window, bin) group counts maxed over
    cores, plus derived col/offset maps (identical on all cores)."""
    maxg = np.zeros((NWIN, NBIN), np.int64)
    per_core = []
    for c in range(N_CORES):
        m = (dst >= c * NL) & (dst < (c + 1) * NL)
        es = src[m]
        ed = dst[m] - c * NL
        o = np.argsort(ed, kind='stable')
        es, ed = es[o], ed[o]
        w = ed // WIN
        b = es >> 15
        cnt = np.zeros((NWIN, NBIN), np.int64)
        np.add.at(cnt, (w, b), 1)
        maxg = np.maximum(maxg, -(-cnt // P))
        per_core.append((es, ed, w, b))
    G = maxg  # [NWIN, NBIN] groups
    NG_w = G.sum(1)                       # uses per window
    GTOT = int(NG_w.sum())
    roff = np.zeros(NWIN + 1, np.int64)   # rel/oht col offset per window
    roff[1:] = np.cumsum(NG_w)
    # per batch: gt col layout (bin-major, window, group) + idx offsets
    gcol = {}      # (w, b) -> first gt col of that window/bin's groups
    goff = {}      # (B, b) -> first gt col of bin block
    nidx = {}      # (B, b) -> num idxs
    xoff = {}      # (B, b) -> idx col offset (int16 cols)
    NGB = np.zeros(NBATCH, np.int64)
    xo = 0
    for B in range(NBATCH):
        col = 0
        for b in range(NBIN):
            goff[(B, b)] = col
            n = 0
            for w in range(B * WB, (B + 1) * WB):
                gcol[(w, b)] = col
                col += int(G[w, b])
                n += int(G[w, b]) * P
            nidx[(B, b)] = n
            xoff[(B, b)] = xo
            xo += n // 16
        NGB[B] = col
    return dict(G=G, NG_w=NG_w, GTOT=GTOT, roff=roff, gcol=gcol, goff=goff,
                nidx=nidx, xoff=xoff, NGB=NGB, XTOT=xo, per_core=per_core)


def prep_core(st, core):
    """Per-core idx16 / rel / oht8 arrays following the global structure."""
    es, ed, w_arr, b_arr = st['per_core'][core]
    G, roff = st['G'], st['roff']
    GTOT, XTOT = st['GTOT'], st['XTOT']
    idx16 = np.zeros((P, XTOT), np.int16)
    rel = np.full((P, GTOT), 255.0, dtype=BF)
    oht = np.zeros((64, GTOT * P), np.uint8)
    # bucket edges per (w, b)
    order = np.lexsort((b_arr,))  # stable by bin, edges already dst-sorted
    # simpler: build lists per (w, b)
    key = w_arr * NBIN + b_arr
    ksort = np.argsort(key, kind='stable')
    es_s, ed_s = es[ksort], ed[ksort]
    key_s = key[ksort]
    bounds = np.searchsorted(key_s, np.arange(NWIN * NBIN + 1))
    wl = np.arange(WIN)
    for B in range(NBATCH):
        for b in range(NBIN):
            flat = []
            for w in range(B * WB, (B + 1) * WB):
                g = int(G[w, b])
                if g == 0:
                    continue
                lo, hi = bounds[w * NBIN + b], bounds[w * NBIN + b + 1]
                e_src = es_s[lo:hi] - (b << 15)
                e_rel = ed_s[lo:hi] - w * WIN
                nslot = g * P
                pad = nslot - (hi - lo)
                vals = np.concatenate([e_src, np.zeros(pad, np.int64)])
                rels = np.concatenate([e_rel, np.full(pad, 255, np.int64)])
                flat.append(vals)
                # rel cols: window-major, u = (b-major offset) within window
                u0 = int(roff[w] + G[w, :b].sum())
                r2 = rels.reshape(g, P)
                rel[:, u0:u0 + g] = r2.T.astype(BF)
                # oht[wl, (u, p)] = (rel == wl)
                blk = (r2[:, None, :] == wl[None, :, None])  # [g, 64, P]
                oht[:, (u0) * P:(u0 + g) * P] = (
                    blk.transpose(1, 0, 2).reshape(64, g * P).astype(np.uint8))
            if not flat:
                continue
            fv = np.concatenate(flat).astype(np.int16)
            wrapped = fv.reshape(-1, 16).T  # [16, n/16]
            xo = st['xoff'][(B, b)]
            idx16[:, xo:xo + wrapped.shape[1]] = np.tile(wrapped, (8, 1))
    return idx16, rel, oht


def make_wcat(W, a_src, a_dst):
    """[fin, 128] bf16: cols 0:64 W^T | 64:66 W^T@Asrc | 66 zero(ones slot)
    | 67:69 W^T@Adst | rest zero."""
    W = np.asarray(W, np.float64)
    fin = W.shape[1]
    As = np.zeros((HC, 2)); Ad = np.zeros((HC, 2))
    a_src = np.asarray(a_src, np.float64).reshape(H, C)
    a_dst = np.asarray(a_dst, np.float64).reshape(H, C)
    for h in range(H):
        As[h * C:(h + 1) * C, h] = a_src[h]
        Ad[h * C:(h + 1) * C, h] = a_dst[h]
    out = np.zeros((fin, EC), np.float64)
    out[:, 0:HC] = W.T
    out[:, 64:66] = W.T @ As
    out[:, 67:69] = W.T @ Ad
    return out.astype(BF)


def prep_all(inputs):
    ei = np.asarray(inputs['edge_index']).astype(np.int64)
    loops = np.arange(N_FULL, dtype=np.int64)
    src = np.concatenate([ei[0], loops])
    dst = np.concatenate([ei[1], loops])
    st = build_structure(src, dst)
    batch = np.asarray(inputs['batch']).astype(np.int64)
    counts = np.bincount(batch, minlength=G_FULL).astype(np.float32).reshape(G_FULL, 1)
    x = np.asarray(inputs['x']).astype(np.float32)
    cores = []
    for c in range(N_CORES):
        idx16, rel, oht = prep_core(st, c)
        xT = np.zeros((F_IN, NLP), dtype=BF)
        xT[:, :NL] = x[c * NL:(c + 1) * NL].T.astype(BF)
        b_loc = batch[c * NL:(c + 1) * NL]
        pg = np.zeros((P, NCH * G_FULL), dtype=BF)
        for ch in range(NCH):
            bb = b_loc[ch * P:(ch + 1) * P]
            pg[np.arange(len(bb)), ch * G_FULL + bb] = 1.0
        cores.append(dict(xT=xT, idx=idx16, rel=rel, oht8=oht, pg=pg))
    meta = dict(
        st=st, counts=counts,
        Wcat=[make_wcat(inputs['W0'], inputs['a_src0'], inputs['a_dst0']),
              make_wcat(inputs['W1'], inputs['a_src1'], inputs['a_dst1']),
              make_wcat(inputs['W2'], inputs['a_src2'], inputs['a_dst2'])],
        mlp_w1T=np.ascontiguousarray(np.asarray(inputs['mlp_w1']).T).astype(np.float32),
        mlp_w2T=np.ascontiguousarray(np.asarray(inputs['mlp_w2']).T).astype(np.float32),
        b1rep=np.tile(np.asarray(inputs['mlp_b1']).astype(np.float32)[None, :], (G_FULL, 1)),
        b2rep=np.tile(np.asarray(inputs['mlp_b2']).astype(np.float32)[None, :], (G_FULL, 1)),
    )
    for l in range(3):
        assert np.all(np.asarray(inputs[f'b{l}']) == 0), "nonzero GAT bias unsupported"
    return cores, meta


# ---------------------------------------------------------------- program

def build_prog(meta):
    st = meta['st']
    G, NG_w, roff = st['G'], st['NG_w'], st['roff']
    GTOT, XTOT, NGB = st['GTOT'], st['XTOT'], st['NGB']
    NGBmax = int(NGB.max())
    NGWmax = int(NG_w.max())
    XBmax = max(st['xoff'][(B, NBIN - 1)] + st['nidx'][(B, NBIN - 1)] // 16
                - st['xoff'][(B, 0)] for B in range(NBATCH))
    NG4max = max(int(roff[(B + 1) * WB] - roff[B * WB]) for B in range(NBATCH))
    AF = mybir.ActivationFunctionType
    OP = mybir.AluOpType

    nc = bacc.Bacc("TRN2", target_bir_lowering=False, debug=False,
                   num_devices=N_CORES)
    t_xT = nc.dram_tensor("xT", [F_IN, NLP], BF16, kind="ExternalInput")
    t_idx = nc.dram_tensor("idx", [P, XTOT], I16, kind="ExternalInput")
    t_rel = nc.dram_tensor("rel", [P, GTOT], BF16, kind="ExternalInput")
    t_oht = nc.dram_tensor("oht8", [64, GTOT * P], U8, kind="ExternalInput")
    t_wc = [nc.dram_tensor(f"Wcat{l}", [F_IN if l == 0 else HC, EC], BF16,
                           kind="ExternalInput") for l in range(3)]
    t_pg = nc.dram_tensor("pg", [P, NCH * G_FULL], BF16, kind="ExternalInput")
    t_cnt = nc.dram_tensor("counts", [G_FULL, 1], F32, kind="ExternalInput")
    t_w1T = nc.dram_tensor("mlp_w1T", [HC, 32], F32, kind="ExternalInput")
    t_w2T = nc.dram_tensor("mlp_w2T", [32, 2], F32, kind="ExternalInput")
    t_b1 = nc.dram_tensor("b1rep", [G_FULL, 32], F32, kind="ExternalInput")
    t_b2 = nc.dram_tensor("b2rep", [G_FULL, 2], F32, kind="ExternalInput")
    t_out = nc.dram_tensor("out", [G_FULL, 2], F32, kind="ExternalOutput")

    with TileContext(nc) as tc:
        from concourse.masks import make_identity
        with ExitStack() as ctx:
            const = ctx.enter_context(tc.tile_pool(name="const", bufs=1))
            dram1 = ctx.enter_context(tc.tile_pool(name="dram1", bufs=1, space="DRAM"))
            dram2 = ctx.enter_context(tc.tile_pool(name="dram2", bufs=2, space="DRAM"))
            dram3 = ctx.enter_context(tc.tile_pool(name="dram3", bufs=3, space="DRAM"))
            p_xb = ctx.enter_context(tc.tile_pool(name="p_xb", bufs=2))
            p_xt = ctx.enter_context(tc.tile_pool(name="p_xt", bufs=2))
            p_ts = ctx.enter_context(tc.tile_pool(name="p_ts", bufs=2))
            p_gt = ctx.enter_context(tc.tile_pool(name="p_gt", bufs=2))
            p_ix = ctx.enter_context(tc.tile_pool(name="p_ix", bufs=2))
            p_oh = ctx.enter_context(tc.tile_pool(name="p_oh", bufs=2))
            p_ml = ctx.enter_context(tc.tile_pool(name="p_ml", bufs=2))
            p_s2 = ctx.enter_context(tc.tile_pool(name="p_s2", bufs=2))
            p_ew = ctx.enter_context(tc.tile_pool(name="p_ew", bufs=2))
            p_st = ctx.enter_context(tc.tile_pool(name="p_st", bufs=2))
            p_ep = ctx.enter_context(tc.tile_pool(name="p_ep", bufs=2))
            p_ad = ctx.enter_context(tc.tile_pool(name="p_ad", bufs=2))
            p_mi = ctx.enter_context(tc.tile_pool(name="p_mi", bufs=3))
            ps_b = ctx.enter_context(tc.tile_pool(name="ps_b", bufs=1, space="PSUM"))
            ps_a = ctx.enter_context(tc.tile_pool(name="ps_a", bufs=2, space="PSUM"))
            ps_n = ctx.enter_context(tc.tile_pool(name="ps_n", bufs=2, space="PSUM"))
            ps_m = ctx.enter_context(tc.tile_pool(name="ps_m", bufs=1, space="PSUM"))

            identb = const.tile([P, P], BF16)
            make_identity(nc, identb[:])
            identf = const.tile([P, P], F32)
            make_identity(nc, identf[:])
            ones1b = const.tile([P, 1], BF16)
            nc.vector.memset(ones1b[:], 1.0)
            iota_i = const.tile([P, WIN], mybir.dt.int32)
            nc.gpsimd.iota(iota_i[:], pattern=[[1, WIN]], base=0, channel_multiplier=0)
            iotab = const.tile([P, WIN], BF16)
            nc.vector.tensor_copy(iotab[:], iota_i[:])

            xn_cur = None

            for l in range(3):
                fin = F_IN if l == 0 else HC
                Wcb = const.tile([fin, EC], BF16, tag=f"wc{l}")
                nc.sync.dma_start(out=Wcb[:], in_=t_wc[l].ap())
                slice_t = dram2.tile([NL, EC], BF16, tag="slice")
                adloc = dram2.tile([NLP, 2], BF16, tag="adloc")

                # ---- table build ----
                for c8 in range(0, NCH, 8):
                    ncb = min(8, NCH - c8)
                    if l == 0:
                        xb = p_xb.tile([fin, 8 * P], BF16, tag="xb0")
                        nc.sync.dma_start(
                            out=xb[:, 0:ncb * P],
                            in_=t_xT.ap()[:, c8 * P:(c8 + ncb) * P])
                    else:
                        xb = p_xb.tile([P, 8, HC], BF16, tag="xbm")
                        nc.sync.dma_start(
                            out=xb[:, 0:ncb, :],
                            in_=xn_cur[c8 * P:(c8 + ncb) * P, :]
                                .rearrange("(k p) f -> p k f", p=P))
                    for c in range(c8, c8 + ncb):
                        pc = min(P, NL - c * P)
                        j = c % 4
                        if l == 0:
                            lhsT = xb[:, (c - c8) * P:(c - c8) * P + pc]
                        else:
                            pst = ps_b.tile([HC, P], BF16, tag="pstr")
                            nc.tensor.transpose(out=pst[:], in_=xb[:, c - c8, :],
                                                identity=identb[:])
                            xtc = p_xt.tile([HC, P], BF16, tag="xtc")
                            nc.scalar.copy(xtc[:], pst[:])
                            lhsT = xtc[:, 0:pc]
                        ps_t = ps_b.tile([P, EC], F32, tag="ps_t")
                        nc.tensor.matmul(out=ps_t[:pc], lhsT=lhsT, rhs=Wcb[:],
                                         start=True, stop=True)
                        if j == 0:
                            tstage = p_ts.tile([P, 4, EC], BF16, tag="tstage")
                        if pc < P:
                            nc.vector.memset(tstage[:, j, :], 0.0)
                        nc.scalar.copy(tstage[:pc, j, :], ps_t[:pc])
                        nc.vector.tensor_copy(tstage[:pc, j, 66:67], ones1b[:pc])
                        if j == 3 or c == NCH - 1:
                            kb = j + 1
                            r0 = (c - j) * P
                            kfull = kb if pc == P else kb - 1
                            if kfull > 0:
                                nc.sync.dma_start(
                                    out=slice_t[r0:r0 + kfull * P, :]
                                        .rearrange("(k p) c -> p k c", p=P),
                                    in_=tstage[:, 0:kfull, :])
                            if pc < P:
                                nc.sync.dma_start(
                                    out=slice_t[r0 + kfull * P:r0 + kfull * P + pc, :],
                                    in_=tstage[:pc, kfull, :])
                            nc.sync.dma_start(
                                out=adloc[r0:r0 + kb * P, :]
                                    .rearrange("(k p) c -> p k c", p=P),
                                in_=tstage[:, 0:kb, 67:69])
                table = dram3.tile([N_FULL, EC], BF16, tag="table")
                nc.gpsimd.collective_compute(
                    "AllGather", OP.bypass,
                    replica_groups=[list(range(N_CORES))],
                    ins=[slice_t.opt()], outs=[table.opt()])

                # ---- windows ----
                xn_new = dram2.tile([NLP, HC], BF16, tag="xn")
                for B in range(NBATCH):
                    xlo = st['xoff'][(B, 0)]
                    xhi = st['xoff'][(B, NBIN - 1)] + st['nidx'][(B, NBIN - 1)] // 16
                    idxb = p_ix.tile([P, XBmax], I16, tag="idxb")
                    nc.sync.dma_start(out=idxb[:, 0:xhi - xlo], in_=t_idx.ap()[:, xlo:xhi])
                    rlo, rhi = int(roff[B * WB]), int(roff[(B + 1) * WB])
                    relb = p_ix.tile([P, NG4max], BF16, tag="relb")
                    nc.sync.dma_start(out=relb[:, 0:rhi - rlo], in_=t_rel.ap()[:, rlo:rhi])
                    ohtc = p_oh.tile([64, NG4max * P], U8, tag="ohtc")
                    nc.sync.dma_start(out=ohtc[:, 0:(rhi - rlo) * P],
                                      in_=t_oht.ap()[:, rlo * P:rhi * P])
                    gt = p_gt.tile([P, NGBmax, EC], BF16, tag="gt")
                    adwb = p_ad.tile([64, WB, 2], BF16, tag="adw")
                    nc.sync.dma_start(
                        out=adwb[:],
                        in_=adloc[B * WB * WIN:(B + 1) * WB * WIN, :]
                            .rearrange("(k wl) c -> wl k c", wl=WIN))
                    for b in range(NBIN):
                        n = st['nidx'][(B, b)]
                        if n == 0:
                            continue
                        g0 = st['goff'][(B, b)]
                        xo = st['xoff'][(B, b)] - xlo
                        nc.gpsimd.dma_gather(
                            out_ap=gt[:, g0:g0 + n // P, :],
                            in_ap=table[b * BINSZ:min((b + 1) * BINSZ, N_FULL), :],
                            idxs_ap=idxb[:, xo:xo + n // 16],
                            num_idxs=n, num_idxs_reg=n, elem_size=EC,
                            single_packet=False)
                    for w in range(B * WB, (B + 1) * WB):
                        ng = int(NG_w[w])
                        u0 = int(roff[w]) - rlo
                        ohtf = p_oh.tile([64, NGWmax * P], BF16, tag="ohtf")
                        nc.scalar.copy(ohtf[:, 0:ng * P], ohtc[:, u0 * P:(u0 + ng) * P])
                        mall = p_ml.tile([P, NGWmax, WIN], BF16, tag="mall")
                        nc.vector.tensor_tensor(
                            out=mall[:, 0:ng, :],
                            in0=iotab[:].rearrange("p (u w) -> p u w", u=1)
                                .to_broadcast([P, ng, WIN]),
                            in1=relb[:, u0:u0 + ng]
                                .rearrange("p (g u) -> p g u", u=1)
                                .to_broadcast([P, ng, WIN]),
                            op=OP.is_equal)
                        psad = ps_a.tile([P, NGWmax, 2], F32, tag="psad")
                        for u in range(ng):
                            nc.tensor.matmul(
                                out=psad[:, u, :],
                                lhsT=ohtf[:, u * P:(u + 1) * P],
                                rhs=adwb[:, w - B * WB, :],
                                start=True, stop=True)
                        ew = p_ew.tile([P, NGWmax, 2], F32, tag="ew")
                        for b in range(NBIN):
                            gb = int(G[w, b])
                            if gb == 0:
                                continue
                            ub = int(G[w, :b].sum())
                            gc = st['gcol'][(w, b)]
                            nc.vector.tensor_tensor(
                                out=ew[:, ub:ub + gb, :],
                                in0=gt[:, gc:gc + gb, 64:66],
                                in1=psad[:, ub:ub + gb, :], op=OP.add)
                        ew2 = p_ew.tile([P, NGWmax * 2], F32, tag="ew2")
                        nc.vector.tensor_scalar_mul(
                            ew2[:, 0:ng * 2],
                            ew[:, 0:ng, :].rearrange("p g h -> p (g h)"), NEG)
                        nc.vector.tensor_tensor(
                            out=ew[:, 0:ng, :].rearrange("p g h -> p (g h)"),
                            in0=ew[:, 0:ng, :].rearrange("p g h -> p (g h)"),
                            in1=ew2[:, 0:ng * 2], op=OP.max)
                        ewb = p_ew.tile([P, NGWmax, 2], BF16, tag="ewb")
                        nc.scalar.activation(
                            ewb[:, 0:ng, :].rearrange("p g h -> p (g h)"),
                            ew[:, 0:ng, :].rearrange("p g h -> p (g h)"), AF.Exp)
                        S2 = p_s2.tile([P, NGWmax, 2 * WIN], BF16, tag="S2")
                        nc.vector.tensor_tensor(
                            out=S2[:, 0:ng, :].rearrange("p g (h w) -> p g h w", h=2),
                            in0=mall[:, 0:ng, :].rearrange("p g (u w) -> p g u w", u=1)
                                .to_broadcast([P, ng, 2, WIN]),
                            in1=ewb[:, 0:ng, :].rearrange("p g (h u) -> p g h u", u=1)
                                .to_broadcast([P, ng, 2, WIN]),
                            op=OP.mult)
                        psn = ps_n.tile([P, 67], F32, tag="psn")
                        u = 0
                        for b in range(NBIN):
                            gb = int(G[w, b])
                            gc = st['gcol'][(w, b)]
                            for g in range(gb):
                                nc.tensor.matmul(
                                    out=psn[:],
                                    lhsT=S2[:, u, :], rhs=gt[:, gc + g, 0:67],
                                    start=(u == 0), stop=(u == ng - 1))
                                u += 1
                        kk = w % KEPI
                        if kk == 0:
                            stage = p_st.tile([P, KEPI, 67], F32, tag="stage")
                        nc.scalar.copy(stage[:, kk, :], psn[:])
                        if kk == KEPI - 1:
                            w0 = w - (KEPI - 1)
                            den = p_ep.tile([P, KEPI, 1], F32, tag="den")
                            nc.vector.tensor_scalar_add(den[:], stage[:, :, 66:67], 1e-16)
                            rec = p_ep.tile([P, KEPI, 1], F32, tag="rec")
                            nc.vector.reciprocal(rec[:], den[:])
                            hv = p_ep.tile([P, KEPI, HC], F32, tag="hv")
                            nc.vector.tensor_tensor(
                                out=hv[:], in0=stage[:, :, 0:HC],
                                in1=rec[:].to_broadcast([P, KEPI, HC]), op=OP.mult)
                            t1 = p_ep.tile([P, KEPI * HC], F32, tag="t1")
                            nc.vector.tensor_scalar_max(
                                t1[:], hv[:].rearrange("p k c -> p (k c)"), 0.0)
                            t2 = p_ep.tile([P, KEPI * HC], F32, tag="t2")
                            nc.vector.tensor_scalar_min(
                                t2[:], hv[:].rearrange("p k c -> p (k c)"), 0.0)
                            nc.scalar.activation(t2[:], t2[:], AF.Exp)
                            nc.vector.tensor_tensor(out=t1[:], in0=t1[:], in1=t2[:],
                                                    op=OP.add)
                            elub = p_ep.tile([P, KEPI, HC], BF16, tag="elub")
                            nc.vector.tensor_scalar_add(
                                elub[:].rearrange("p k c -> p (k c)"), t1[:], -1.0)
                            for h in (0, 1):
                                nc.sync.dma_start(
                                    out=xn_new[w0 * WIN:(w0 + KEPI) * WIN,
                                               32 * h:32 * h + 32]
                                        .rearrange("(k wl) c -> wl k c", wl=WIN),
                                    in_=elub[WIN * h:WIN * h + WIN, :,
                                             32 * h:32 * h + 32])
                xn_cur = xn_new

            # ---- pool + MLP ----
            pgt = p_mi.tile([P, NCH * G_FULL], BF16, tag="pgt")
            nc.sync.dma_start(out=pgt[:], in_=t_pg.ap())
            psg = ps_m.tile([G_FULL, HC], F32, tag="psg")
            for c8 in range(0, NCH, 8):
                ncb = min(8, NCH - c8)
                xcb = p_xb.tile([P, 8, HC], BF16, tag="xcb")
                nc.sync.dma_start(
                    out=xcb[:, 0:ncb, :],
                    in_=xn_cur[c8 * P:(c8 + ncb) * P, :]
                        .rearrange("(k p) f -> p k f", p=P))
                for c in range(c8, c8 + ncb):
                    nc.tensor.matmul(
                        out=psg[:], lhsT=pgt[:, c * G_FULL:(c + 1) * G_FULL],
                        rhs=xcb[:, c - c8, :],
                        start=(c == 0), stop=(c == NCH - 1))
            pool_l = dram1.tile([G_FULL, HC], F32)
            pool_s = p_mi.tile([G_FULL, HC], F32, tag="pool_s")
            nc.scalar.copy(pool_s[:], psg[:])
            nc.sync.dma_start(out=pool_l[:], in_=pool_s[:])
            pool_r = dram1.tile([G_FULL, HC], F32)
            nc.gpsimd.collective_compute(
                "AllReduce", mybir.AluOpType.add,
                replica_groups=[list(range(N_CORES))],
                ins=[pool_l.opt()], outs=[pool_r.opt()])
            pooled = p_mi.tile([G_FULL, HC], F32, tag="pooled")
            nc.sync.dma_start(out=pooled[:], in_=pool_r[:])
            cnt = p_mi.tile([G_FULL, 1], F32, tag="cnt")
            nc.sync.dma_start(out=cnt[:], in_=t_cnt.ap())
            nc.vector.tensor_scalar_max(cnt[:], cnt[:], 1.0)
            rc = p_mi.tile([G_FULL, 1], F32, tag="rc")
            nc.vector.reciprocal(out=rc[:], in_=cnt[:])
            nc.vector.tensor_tensor(out=pooled[:], in0=pooled[:],
                                    in1=rc[:].to_broadcast([G_FULL, HC]), op=mybir.AluOpType.mult)
            ps_pt = ps_m.tile([HC, G_FULL], F32, tag="mlp_ps")
            nc.tensor.transpose(out=ps_pt[:], in_=pooled[:],
                                identity=identf[0:G_FULL, 0:G_FULL])
            poolT = p_mi.tile([HC, G_FULL], F32, tag="poolT")
            nc.scalar.copy(poolT[:], ps_pt[:])
            w1 = p_mi.tile([HC, 32], F32, tag="w1")
            nc.sync.dma_start(out=w1[:], in_=t_w1T.ap())
            ps_z = ps_m.tile([G_FULL, 32], F32, tag="mlp_ps")
            nc.tensor.matmul(out=ps_z[:], lhsT=poolT[:], rhs=w1[:], start=True, stop=True)
            z1 = p_mi.tile([G_FULL, 32], F32, tag="z1")
            b1t = p_mi.tile([G_FULL, 32], F32, tag="b1t")
            nc.sync.dma_start(out=b1t[:], in_=t_b1.ap())
            nc.vector.tensor_tensor(out=z1[:], in0=ps_z[:], in1=b1t[:], op=mybir.AluOpType.add)
            nc.scalar.activation(z1[:], z1[:], AF.Relu)
            ps_zt = ps_m.tile([32, G_FULL], F32, tag="mlp_ps")
            nc.tensor.transpose(out=ps_zt[:], in_=z1[:],
                                identity=identf[0:G_FULL, 0:G_FULL])
            z1T = p_mi.tile([32, G_FULL], F32, tag="z1T")
            nc.scalar.copy(z1T[:], ps_zt[:])
            w2 = p_mi.tile([32, 2], F32, tag="w2")
            nc.sync.dma_start(out=w2[:], in_=t_w2T.ap())
            ps_o = ps_m.tile([G_FULL, 2], F32, tag="mlp_ps")
            nc.tensor.matmul(out=ps_o[:], lhsT=z1T[:], rhs=w2[:], start=True, stop=True)
            b2t = p_mi.tile([G_FULL, 2], F32, tag="b2t")
            nc.sync.dma_start(out=b2t[:], in_=t_b2.ap())
            outt = p_mi.tile([G_FULL, 2], F32, tag="outt")
            nc.vector.tensor_tensor(out=outt[:], in0=ps_o[:], in1=b2t[:], op=mybir.AluOpType.add)
            nc.sync.dma_start(out=t_out.ap(), in_=outt[:])
    nc.compile()
    return nc


def run(inputs, trace=False):
    cores, meta = prep_all(inputs)
    nc = build_prog(meta)
    ims = []
    for c in range(N_CORES):
        d = cores[c]
        ims.append(dict(
            xT=d['xT'], idx=d['idx'], rel=d['rel'], oht8=d['oht8'], pg=d['pg'],
            Wcat0=meta['Wcat'][0], Wcat1=meta['Wcat'][1], Wcat2=meta['Wcat'][2],
            counts=meta['counts'], mlp_w1T=meta['mlp_w1T'],
            mlp_w2T=meta['mlp_w2T'], b1rep=meta['b1rep'], b2rep=meta['b2rep']))
    res = run_bass_kernel_spmd(nc, ims, core_ids=list(range(N_CORES)), trace=trace)
    return np.asarray(res.results[0]['out']), res


def kernel(**inputs):
    import os
    trace = bool(os.environ.get("GAT_TRACE"))
    out, res = run(inputs, trace=trace)
    if trace:
        kernel.last_exec_ns = res.exec_time_ns
    return np.asarray(out, dtype=np.float32)
